# revision 17
# baseline (speedup 1.0000x reference)
"""Trainium2 Bass kernel for CapsuleLayer (dynamic routing), 8-core data-parallel.

Problem: x [128, 1152, 512] f32, W [512, 160] f32.
  u_hat = (x @ W).reshape(B, N, 10, 16)
  b = 0; 3 routing iterations of softmax/weighted-sum/squash.
Output: v [128, 10, 16] f32.

Sharding: data-parallel over batch. Each of the 8 cores gets 16 batches
(x shard [16*1152, 512]) and the full W; no cross-core communication.

Per-core pipeline:
  Phase 1 (streamed over 144 row-tiles of 128):
    - DMA x tile f32 -> SBUF, cast to bf16 (DVE or SWDGE cast-DMA)
    - transpose each [128,128] block on the PE as a *regular* bf16 matmul
      against an identity moving operand (full speed, unlike transpose-mode)
    - u_hat tile = xT.T @ W accumulated over the 4 k-chunks in PSUM,
      copied to SBUF as bf16 in [n, 160] layout (natural for routing)
  Phase 2 (on-chip routing, u_hat resident in SBUF):
    - capsule sums s: wide DVE tree-adds over the 9 tiles of each batch,
      then a single ones-column matmul for the 128-partition reduction
    - softmax over capsules without max-subtraction (|b| stays small)
    - squash per batch on [1,160] tiles; b update via one whole-core
      multiply + grouped reduce
"""

import os
import sys

import numpy as np

sys.path.insert(0, "/opt/trn_rl_repo")

import concourse.bass as bass
import concourse.tile as tile
import concourse.mybir as mybir
from concourse import bacc

F32 = mybir.dt.float32
BF16 = mybir.dt.bfloat16

B, N, K, C, D = 128, 1152, 512, 10, 16
CD = C * D  # 160
NCORES = 8
BSH = B // NCORES  # batches per core

USE_SWDGE_CAST = True  # cast f32->bf16 inside the gpsimd DMA
USE_DMA_TRANSPOSE = False  # xbar SBUF->SBUF transpose instead of PE transpose


def build_core_program(bsh=BSH, n_len=N, nc=None):
    """Build the single-core Bass program for a shard of `bsh` batches."""
    assert n_len % 128 == 0
    tpb = n_len // 128  # row-tiles per batch
    nt = bsh * tpb  # total row-tiles
    rows = bsh * n_len

    if nc is None:
        nc = bacc.Bacc("TRN2", target_bir_lowering=False, debug=False)

    x_in = nc.declare_dram_parameter("x", [rows, K], F32, isOutput=False).ap()
    w_in = nc.declare_dram_parameter("W", [K, CD], F32, isOutput=False).ap()
    id_in = nc.declare_dram_parameter("ident", [128, 128], BF16, isOutput=False).ap()
    v_out = nc.declare_dram_parameter("v", [bsh, CD], F32, isOutput=True).ap()

    with tile.TileContext(nc) as tc:
        _build_body(tc, x_in, w_in, id_in, v_out, bsh, tpb)
    nc.finalize()
    return nc


def _build_body(tc, x_in, w_in, id_in, v_out, bsh, tpb):
    nc = tc.nc
    nt = bsh * tpb
    KT = K // 128  # 4 contraction chunks

    from contextlib import ExitStack

    with ExitStack() as ctx:
        singles = ctx.enter_context(tc.tile_pool(name="singles", bufs=1))
        persist = ctx.enter_context(tc.tile_pool(name="persist", bufs=1))
        pool_x = ctx.enter_context(tc.tile_pool(name="xf", bufs=3))
        pool_xb = ctx.enter_context(tc.tile_pool(name="xb", bufs=3))
        pool_xT = ctx.enter_context(tc.tile_pool(name="xT", bufs=6))
        pool_sm = ctx.enter_context(tc.tile_pool(name="smalls", bufs=6))
        ps_T = ctx.enter_context(tc.tile_pool(name="psT", bufs=3, space="PSUM"))
        ps_U = ctx.enter_context(tc.tile_pool(name="psU", bufs=2, space="PSUM"))
        ps_S = ctx.enter_context(tc.tile_pool(name="psS", bufs=2, space="PSUM"))

        # --- constants ---
        ident = singles.tile([128, 128], BF16)
        nc.sync.dma_start(out=ident, in_=id_in)
        # all-ones / all-0.1 stationary operands: the capsule-sum matmuls use
        # M=128 so the (identical) row sums land replicated on all partitions,
        # which lets the squash chain and the b-update run without any
        # partition-broadcast (illegal on DVE).
        ones_m = singles.tile([128, 128], BF16)
        nc.vector.memset(ones_m, 1.0)
        tenth_m = singles.tile([128, 128], BF16)
        nc.vector.memset(tenth_m, 1.0 / C)

        w_f32 = singles.tile([128, KT, CD], F32)
        nc.sync.dma_start(out=w_f32, in_=w_in.rearrange("(j p) c -> p j c", p=128))
        w_bf = singles.tile([128, KT, CD], BF16)
        nc.vector.tensor_copy(w_bf, w_f32)

        # --- persistent tensors ---
        u_hat = persist.tile([128, nt, CD], BF16)
        w_scr = persist.tile([128, nt, CD], BF16)
        b_log = persist.tile([128, nt * C], F32)
        b_tmp = persist.tile([128, nt * C], F32)
        e_exp = persist.tile([128, nt * C], F32)
        c_sm = persist.tile([128, nt * C], BF16)
        ssum = persist.tile([128, nt], F32)
        acc4 = persist.tile([128, bsh, 4, CD], BF16)
        accf = persist.tile([128, bsh, CD], BF16)
        vrep = persist.tile([128, bsh, CD], F32)  # v, replicated on partitions

        # ---------------- Phase 1: u_hat = x @ W ----------------
        for t in range(nt):
            if USE_SWDGE_CAST:
                xb = pool_xb.tile([128, K], BF16, tag="xb")
                nc.gpsimd.dma_start(
                    out=xb, in_=x_in[t * 128 : (t + 1) * 128, :]
                )
            else:
                xf = pool_x.tile([128, K], F32, tag="xf")
                nc.sync.dma_start(out=xf, in_=x_in[t * 128 : (t + 1) * 128, :])
                xb = pool_xb.tile([128, K], BF16, tag="xb")
                nc.vector.tensor_copy(xb, xf)

            xts = []
            for j in range(KT):
                xt = pool_xT.tile([128, 128], BF16, tag="xT")
                if USE_DMA_TRANSPOSE:
                    nc.sync.dma_start_transpose(xt, xb[:, j * 128 : (j + 1) * 128])
                else:
                    pt = ps_T.tile([128, 128], F32, tag="psT")
                    # regular matmul against identity: out = xb_j.T (full speed)
                    nc.tensor.matmul(
                        pt,
                        lhsT=xb[:, j * 128 : (j + 1) * 128],
                        rhs=ident,
                        start=True,
                        stop=True,
                    )
                    nc.scalar.copy(xt, pt)  # PSUM f32 -> SBUF bf16
                xts.append(xt)

            pu = ps_U.tile([128, CD], F32, tag="psU")
            for j in range(KT):
                nc.tensor.matmul(
                    pu,
                    lhsT=xts[j],
                    rhs=w_bf[:, j, :],
                    start=(j == 0),
                    stop=(j == KT - 1),
                )
            nc.vector.tensor_copy(u_hat[:, t, :], pu)

        # ---------------- Phase 2: routing ----------------
        u4 = u_hat[:].rearrange("p (g t) c -> p g t c", g=bsh)
        w4 = w_scr[:].rearrange("p (g t) c -> p g t c", g=bsh)

        def capsule_sums(src4):
            """Tree-reduce the `tpb` tiles of each batch into accf [128,bsh,CD]."""
            if tpb == 9:
                nc.vector.tensor_add(acc4, src4[:, :, 0:4, :], src4[:, :, 4:8, :])
                nc.vector.tensor_add(
                    acc4[:, :, 0:2, :], acc4[:, :, 0:2, :], acc4[:, :, 2:4, :]
                )
                nc.vector.tensor_add(
                    acc4[:, :, 0, :], acc4[:, :, 0, :], acc4[:, :, 1, :]
                )
                nc.vector.tensor_add(accf, acc4[:, :, 0, :], src4[:, :, 8, :])
            else:
                # generic fallback: serial adds
                nc.vector.tensor_copy(accf, src4[:, :, 0, :])
                for j in range(1, tpb):
                    nc.vector.tensor_add(accf, accf, src4[:, :, j, :])

        def squash(g, s_ps, last):
            """vrep[:, g, :] = squash(s); s_ps is PSUM [128, CD] f32, replicated."""
            sq = pool_sm.tile([128, CD], F32, tag="sq")
            nc.scalar.square(sq, s_ps)
            n2 = pool_sm.tile([128, C], F32, tag="n2")
            nc.vector.tensor_reduce(
                n2,
                sq[:].rearrange("p (c d) -> p c d", d=D),
                axis=mybir.AxisListType.X,
                op=mybir.AluOpType.add,
            )
            nrm = pool_sm.tile([128, C], F32, tag="nrm")
            nc.scalar.sqrt(nrm, n2)
            nc.vector.tensor_scalar_add(nrm, nrm, 1e-7)
            fac = pool_sm.tile([128, C], F32, tag="fac")
            nc.vector.tensor_mul(fac, nrm, nrm)
            nc.vector.tensor_scalar_add(fac, fac, 1.0)
            nc.vector.reciprocal(fac, fac)
            nc.vector.tensor_mul(fac, fac, nrm)  # fac = sn/(1+sn^2)
            fb = fac[:].broadcast_to([128, C, D])
            nc.vector.tensor_mul(
                vrep[:, g, :].rearrange("p (c d) -> p c d", d=D),
                s_ps[:].rearrange("p (c d) -> p c d", d=D),
                fb,
            )
            if last:
                nc.sync.dma_start(out=v_out[g : g + 1, :], in_=vrep[0:1, g, :])

        for i in range(3):
            if i == 0:
                capsule_sums(u4)
                lhs = tenth_m  # uniform c = 1/10 folded into the reduction
            else:
                # b update: b += sum_d u_hat * v_prev
                vb = (
                    vrep[:]
                    .broadcast_to([128, bsh, CD, tpb])
                    .rearrange("p g c t -> p g t c")
                )
                nc.vector.tensor_mul(w4, u4, vb)
                tgt = b_log if i == 1 else b_tmp
                nc.vector.tensor_reduce(
                    tgt,
                    w_scr[:].rearrange("p t (c d) -> p (t c) d", d=D),
                    axis=mybir.AxisListType.X,
                    op=mybir.AluOpType.add,
                )
                if i == 2:
                    nc.vector.tensor_add(b_log, b_log, b_tmp)
                # softmax over capsules (no max-subtraction: |b| is small)
                nc.scalar.activation(
                    e_exp, b_log, mybir.ActivationFunctionType.Exp
                )
                nc.vector.tensor_reduce(
                    ssum,
                    e_exp[:].rearrange("p (t c) -> p t c", c=C),
                    axis=mybir.AxisListType.X,
                    op=mybir.AluOpType.add,
                )
                nc.vector.reciprocal(ssum, ssum)
                rb = ssum[:].broadcast_to([128, nt, C])
                nc.vector.tensor_mul(
                    c_sm[:].rearrange("p (t c) -> p t c", c=C),
                    e_exp[:].rearrange("p (t c) -> p t c", c=C),
                    rb,
                )
                # ws = u_hat * c (broadcast over d); reuse w_scr
                cb = (
                    c_sm[:]
                    .rearrange("p (t c) -> p t c", c=C)
                    .broadcast_to([128, nt, C, D])
                )
                nc.vector.tensor_mul(
                    w_scr[:].rearrange("p t (c d) -> p t c d", d=D),
                    u_hat[:].rearrange("p t (c d) -> p t c d", d=D),
                    cb,
                )
                capsule_sums(w4)
                lhs = ones_m

            for g in range(bsh):
                sp = ps_S.tile([128, CD], F32, tag="psS")
                nc.tensor.matmul(sp, lhsT=lhs, rhs=accf[:, g, :], start=True, stop=True)
                squash(g, sp, last=(i == 2))


# ----------------------------------------------------------------------------
_NC_CACHE = {}


def _get_nc():
    key = (BSH, N)
    if key not in _NC_CACHE:
        _NC_CACHE[key] = build_core_program()
    return _NC_CACHE[key]


def _run(x, W, **kw):
    from concourse.bass_utils import run_bass_kernel_spmd

    import ml_dtypes

    nc = _get_nc()
    x = np.ascontiguousarray(x, dtype=np.float32)
    W = np.ascontiguousarray(W, dtype=np.float32)
    ident = np.eye(128, dtype=ml_dtypes.bfloat16)
    shards = x.reshape(NCORES, BSH * N, K)
    in_maps = [{"x": shards[c], "W": W, "ident": ident} for c in range(NCORES)]
    res = run_bass_kernel_spmd(nc, in_maps, core_ids=list(range(NCORES)), **kw)
    v = np.concatenate(
        [res.results[c]["v"].reshape(BSH, C, D) for c in range(NCORES)], axis=0
    )
    return v, res


def kernel(x, W):
    v, _ = _run(x, W)
    return v


def kernel_timed(x, W):
    v, res = _run(x, W, trace=True)
    return v, res.exec_time_ns


def kernel_traced(x, W):
    v, res = _run(x, W, trace=True)
    return v, res
